# revision 23
# baseline (speedup 1.0000x reference)
"""MultiHeadCrossAttention kernel for 8 Trainium2 NeuronCores.

Problem (hardcoded): B=4, Sx=Sy=1024, DIM=1024, H=16, Dh=64, fp32.
  Q = x@W_Qx.T+b_Qx ; K = cat(x@W_Kx.T+b_Kx, y@W_Ky.T+b_Ky) per head
  V = cat(x@W_Vx.T+b_Vx, y@W_Vy.T+b_Vy) ; out = softmax(QK^T/8)V @ W_out.T + b_out

Sharding: core c -> (batch b = c//2, head-group g = c%2 of 8 heads).
Each core computes its batch's attention for its 8 heads plus the partial
out-projection over its 512 features; host sums the two partials per batch
and adds b_out (the "all-reduce after to_out", done in the gather).

Schedule (v2): the ScalarE exp stream (128 x [128,1024] activations ~ 148us)
is the critical path.  Attention blocks start as early as possible (~12us);
all projection work (V, QK for t>=1, out-projection half 0) is emitted as
paced "filler" units inside the attention kt loop so it hides under the
exp stream.  AV matmuls are col-packed two heads per PSUM bank
(tile_position (0,0)/(0,64), M=64 each); the softmax denominator is
accumulated on VectorE (fp32) and turned into a 64-row broadcast via a
ones[128,64] matmul pair, so no ones-column rides the AV matmul.
reciprocal_approx_fast replaces the slow iterative divide.
"""

import os
import sys

os.environ.setdefault("MYCRO_LOCAL_CACHE", "1")
if "/opt/trn_rl_repo" not in sys.path:
    sys.path.insert(0, "/opt/trn_rl_repo")

from collections import deque

import ml_dtypes
import numpy as np

import concourse.bass as bass
import concourse.mybir as mybir
import concourse.tile as tile
from concourse import bass_utils
from concourse.bass_utils import run_bass_kernel_spmd

FP32 = mybir.dt.float32
FP32R = mybir.dt.float32r
BF16 = mybir.dt.bfloat16

DIM = 1024
H = 16          # total heads
HG = 8          # heads per core (head-group)
DH = 64
S = 1024        # Sx = Sy
FS = 512        # feature slice per core (HG * DH)
NCORES = 8

# ---------------------------------------------------------------------------
# harness patches (this snapshot's Tile emits >1 wait per instruction in a
# few places; HW instructions hold one wait)
# ---------------------------------------------------------------------------

def _patched_drain_and_barrier(self, tick_clock, wait_clock):
    from bass_rust import ScopedClock

    nc = self.nc
    drain_inst = nc.sync.drain()
    wait_clock.add_sem_waits(
        drain_inst.ins, ScopedClock({None: tick_clock.global_clock})
    )
    si = drain_inst.ins.sync_info
    waits = list(si.on_wait)
    if len(waits) > 1:
        del si.on_wait[1:]
        for w in waits[1:]:
            nop = nc.sync.nop(nofuse=True, hint="drain_wait_spill")
            if nop.ins.sync_info is None:
                nop.ins.sync_info = mybir.SyncInfo(on_wait=[], on_update=[])
            nop.ins.sync_info.on_wait.append(w)

    nc.all_engine_barrier()
    assert self.sems is not None
    popped = nc._tile_sem_poison_stack.pop()
    assert popped is self._sem_poison
    nc.clear_and_free_semaphores(list(self.sems.allocated().values()))
    nc.all_engine_barrier()


def _spill_excess_waits(nc):
    n = 0
    for fn in nc.m.functions:
        for bb in fn.blocks:
            new_insts = []
            for inst in bb.instructions:
                si = getattr(inst, "sync_info", None)
                cap = 2 if isinstance(inst, mybir.InstEventSemaphore) else 1
                if si is not None and si.on_wait and len(si.on_wait) > cap:
                    extras = list(si.on_wait[cap:])
                    del si.on_wait[cap:]
                    for w in extras:
                        new_insts.append(
                            mybir.InstNoOp(
                                name=f"wspill-{nc.next_id()}",
                                engine=inst.engine,
                                ins=[],
                                outs=[],
                                sync_info=mybir.SyncInfo(on_wait=[w], on_update=[]),
                            )
                        )
                        n += 1
                new_insts.append(inst)
            bb.instructions[:] = new_insts
    return n


tile.TileContext._drain_and_barrier = _patched_drain_and_barrier

if os.environ.get("ENABLE_LDW_OPT") == "1":
    _orig_run_command = bass_utils.run_command

    def _run_command_ldw(argv, **kwargs):
        if isinstance(argv, list):
            argv = ["--enable-ldw-opt=true" if a == "--enable-ldw-opt=false" else a
                    for a in argv]
        return _orig_run_command(argv, **kwargs)

    bass_utils.run_command = _run_command_ldw
bass_utils.upload_artifacts = lambda tmpdir: tmpdir  # no S3 in container


def _register_ntff_hook():
    """Best-effort: enables trace=True runs (used by test harness only)."""
    try:
        import types

        try:
            from antenv.axon_hooks import set_axon_ntff_profile_hook
        except ImportError:
            # this snapshot's antenv lacks axon_hooks; synthesize the
            # two-function hook registry bass_utils expects
            import antenv

            mod = types.ModuleType("antenv.axon_hooks")
            mod._HOOK = None

            def set_axon_ntff_profile_hook(h, _mod=mod):
                _mod._HOOK = h

            def get_axon_ntff_profile_hook(_mod=mod):
                return _mod._HOOK

            mod.set_axon_ntff_profile_hook = set_axon_ntff_profile_hook
            mod.get_axon_ntff_profile_hook = get_axon_ntff_profile_hook
            sys.modules["antenv.axon_hooks"] = mod
            antenv.axon_hooks = mod
        sys.path.insert(0, "/root/.axon_site")
        from trn_agent_boot.trn_boot import _ntff_profile_via_ctypes

        set_axon_ntff_profile_hook(
            _ntff_profile_via_ctypes("/opt/axon/libaxon_pjrt.so")
        )
    except Exception:
        pass


# ---------------------------------------------------------------------------
# device program (identical on all 8 cores; per-core data differs)
# ---------------------------------------------------------------------------

FILLER_MM_PER_SLOT = 4.5   # paced filler emission budget (MMs per exp slot)
POP_START_SLOT = 4         # no filler pops before this global exp slot
P2_BUFS = 22               # softmax-weight tiles buffered in SBUF
P2_TARGET_LIVE = 14        # defer AV flush until this many p2 tiles are live
AV_FLUSH_PER_SLOT = 2      # max AV pairs flushed per slot when over target


def _build_program():
    nc = bass.Bass()

    xT = nc.declare_dram_parameter("xT", [DIM, S], BF16, isOutput=False)
    yT = nc.declare_dram_parameter("yT", [DIM, S], BF16, isOutput=False)
    wq = nc.declare_dram_parameter("wq", [DIM, FS], BF16, isOutput=False)
    wkx = nc.declare_dram_parameter("wkx", [DIM, FS], BF16, isOutput=False)
    wky = nc.declare_dram_parameter("wky", [DIM, FS], BF16, isOutput=False)
    wvx = nc.declare_dram_parameter("wvx", [DIM, FS], BF16, isOutput=False)
    wvy = nc.declare_dram_parameter("wvy", [DIM, FS], BF16, isOutput=False)
    wo = nc.declare_dram_parameter("wo", [FS, DIM], BF16, isOutput=False)
    bq = nc.declare_dram_parameter("bq", [128, 4], FP32, isOutput=False)
    bkx = nc.declare_dram_parameter("bkx", [128, 4], FP32, isOutput=False)
    bky = nc.declare_dram_parameter("bky", [128, 4], FP32, isOutput=False)
    bvx_bc = nc.declare_dram_parameter("bvx_bc", [1, FS], FP32, isOutput=False)
    bvy_bc = nc.declare_dram_parameter("bvy_bc", [1, FS], FP32, isOutput=False)
    outT = nc.declare_dram_parameter("outT", [DIM, S], FP32, isOutput=True)

    EXP = mybir.ActivationFunctionType.Exp

    with tile.TileContext(nc) as tc:
        import contextlib

        with contextlib.ExitStack() as ctx:
            # SBUF pools
            apool = ctx.enter_context(tc.tile_pool(name="apool", bufs=16))
            wpool = ctx.enter_context(tc.tile_pool(name="wpool", bufs=40))
            wopool = ctx.enter_context(tc.tile_pool(name="wopool", bufs=4))
            qkpool = ctx.enter_context(tc.tile_pool(name="qkpool", bufs=12))
            vpool = ctx.enter_context(tc.tile_pool(name="vpool", bufs=16))
            ppool = ctx.enter_context(tc.tile_pool(name="ppool", bufs=P2_BUFS))
            accpool = ctx.enter_context(tc.tile_pool(name="accpool", bufs=2))
            rpool = ctx.enter_context(tc.tile_pool(name="rpool", bufs=3))
            otpool = ctx.enter_context(tc.tile_pool(name="otpool", bufs=4))
            osbpool = ctx.enter_context(tc.tile_pool(name="osbpool", bufs=2))
            cpool = ctx.enter_context(tc.tile_pool(name="cpool", bufs=1))
            # PSUM pools: 2*2 + 2*1 + 1*2 = 8 banks
            scps = ctx.enter_context(tc.tile_pool(name="scps", bufs=2, space="PSUM"))
            avps = ctx.enter_context(tc.tile_pool(name="avps", bufs=2, space="PSUM"))
            pjps = ctx.enter_context(tc.tile_pool(name="pjps", bufs=1, space="PSUM"))

            # ---- constants + ACT table warm-up ----
            ones64 = cpool.tile([128, 64], BF16, tag="ones64")
            nc.vector.memset(ones64[:, :], 1.0)
            dwi = cpool.tile([128, 8], FP32, tag="dwi")
            dwo = cpool.tile([128, 8], BF16, tag="dwo")
            nc.vector.memset(dwi[:, :], 0.0)
            # loads the exp table set (~2.7us) while DMAs stream in
            nc.scalar.activation(out=dwo[:, :], in_=dwi[:, :], func=EXP)
            warm = cpool.tile([128, 128], BF16, tag="warm")
            nc.vector.memset(warm[:, :], 0.0)

            bq_sb = cpool.tile([128, 4], FP32, tag="bq")
            bkx_sb = cpool.tile([128, 4], FP32, tag="bkx")
            bky_sb = cpool.tile([128, 4], FP32, tag="bky")
            bvx_sb = cpool.tile([128, FS], FP32, tag="bvx")
            bvy_sb = cpool.tile([128, FS], FP32, tag="bvy")

            def _bcast_ap(h):
                return bass.AP(
                    tensor=h[:, :].tensor, offset=h[:, :].offset,
                    ap=[[0, 128]] + [list(a) for a in h[:, :].ap[1:]],
                )

            nc.sync.dma_start(out=bq_sb, in_=bq[:, :])
            nc.sync.dma_start(out=bkx_sb, in_=bkx[:, :])
            nc.sync.dma_start(out=bky_sb, in_=bky[:, :])
            nc.gpsimd.dma_start(out=bvx_sb, in_=_bcast_ap(bvx_bc))
            nc.gpsimd.dma_start(out=bvy_sb, in_=_bcast_ap(bvy_bc))

            # ---- big DMAs, in consumption-priority order ----
            def load_rows(dst_pool, dram, n, width, nm, tag, eng=None):
                eng = eng or nc.sync
                out = []
                for i in range(n):
                    t = dst_pool.tile([128, width], BF16, tag=tag, name=f"{nm}{i}")
                    eng.dma_start(out=t, in_=dram[i * 128:(i + 1) * 128, :])
                    out.append(t)
                return out

            # two DMA queues in parallel: weights on sync, activations on
            # gpsimd -- first score needs wq+wkx+xT, which lands ~2x sooner
            xt = load_rows(apool, xT, 8, S, "xt", "act", nc.gpsimd)
            yt = load_rows(apool, yT, 8, S, "yt", "act", nc.gpsimd)
            wq_sb = load_rows(wpool, wq, 8, FS, "wq", "w")
            wkx_sb = load_rows(wpool, wkx, 8, FS, "wkx", "w")
            wky_sb = load_rows(wpool, wky, 8, FS, "wky", "w")
            wvx_sb = load_rows(wpool, wvx, 8, FS, "wvx", "w")
            wvy_sb = load_rows(wpool, wvy, 8, FS, "wvy", "w")
            wo_sb = load_rows(wopool, wo, 4, S, "wo", "wo")

            # ---- persistent result tiles ----
            QT = [qkpool.tile([128, S], BF16, tag="qk", name=f"QT{i}") for i in range(4)]
            KxT = [qkpool.tile([128, S], BF16, tag="qk", name=f"KxT{i}") for i in range(4)]
            KyT = [qkpool.tile([128, S], BF16, tag="qk", name=f"KyT{i}") for i in range(4)]
            V = [vpool.tile([128, HG, DH], BF16, tag="v", name=f"V{i}") for i in range(16)]
            oT = [otpool.tile([128, S], BF16, tag="ot", name=f"oT{i}") for i in range(4)]

            qk_w = [wq_sb, wkx_sb, wky_sb]
            qk_act = [xt, xt, yt]
            qk_bias = [bq_sb, bkx_sb, bky_sb]
            qk_dst = [QT, KxT, KyT]

            # ---------------------------------------------------------------
            # group emitters (each emits a quarter = 4 MMs; q==3 evacuates)
            # ---------------------------------------------------------------
            qk_ps = {}

            def emit_qk_quarter(pi, ft, q):
                key = (pi, ft)
                if key not in qk_ps:
                    qk_ps[key] = pjps.tile(
                        [128, 1024], FP32, tag="pj", name=f"qkps{pi}_{ft}"
                    )
                ps = qk_ps[key]
                for ct in (2 * q, 2 * q + 1):
                    for h2 in range(2):
                        nc.tensor.matmul(
                            ps[:, h2 * 512:(h2 + 1) * 512],
                            qk_w[pi][ct][:, ft * 128:(ft + 1) * 128],
                            qk_act[pi][ct][:, h2 * 512:(h2 + 1) * 512],
                            start=(ct == 0),
                            stop=(ct == 7),
                        )
                if q == 3:
                    nc.vector.tensor_scalar_add(
                        out=qk_dst[pi][ft][:, :],
                        in0=ps[:, :],
                        scalar1=qk_bias[pi][:, ft:ft + 1],
                    )
                    del qk_ps[key]

            v_ps = {}
            v_ready = [False] * 16

            def emit_v_quarter(src_is_y, sg, q):
                key = (src_is_y, sg)
                if key not in v_ps:
                    v_ps[key] = pjps.tile(
                        [128, 1024], FP32, tag="pj", name=f"vps{int(src_is_y)}_{sg}"
                    )
                ps = v_ps[key]
                act = yt if src_is_y else xt
                w_sb = wvy_sb if src_is_y else wvx_sb
                bias_sb = bvy_sb if src_is_y else bvx_sb
                base = 8 if src_is_y else 0
                for ct in (2 * q, 2 * q + 1):
                    for half in range(2):
                        st = 2 * sg + half
                        nc.tensor.matmul(
                            ps[:, half * 512:(half + 1) * 512],
                            act[ct][:, st * 128:(st + 1) * 128],
                            w_sb[ct][:, :],
                            start=(ct == 0),
                            stop=(ct == 7),
                        )
                if q == 3:
                    for half in range(2):
                        st = 2 * sg + half
                        vt = V[base + st]
                        nc.vector.tensor_add(
                            out=vt[:, :, :],
                            in0=ps[:, half * 512:(half + 1) * 512].rearrange(
                                "p (h d) -> p h d", h=HG),
                            in1=bias_sb[:, :].rearrange("p (h d) -> p h d", h=HG),
                        )
                        v_ready[base + st] = True
                    del v_ps[key]

            def emit_op_group(m, half, pool):
                ps = pool.tile([128, 512], FP32,
                               tag="pj" if pool is pjps else "sc",
                               name=f"op{m}_{half}")
                for ft in range(4):
                    nc.tensor.matmul(
                        ps[:, :],
                        wo_sb[ft][:, m * 128:(m + 1) * 128],
                        oT[ft][:, half * 512:(half + 1) * 512],
                        start=(ft == 0),
                        stop=(ft == 3),
                    )
                osb = osbpool.tile([128, 512], FP32, tag="osb", name="osb")
                nc.vector.tensor_copy(out=osb[:, :], in_=ps[:, :])
                nc.sync.dma_start(
                    out=outT[m * 128:(m + 1) * 128, half * 512:(half + 1) * 512],
                    in_=osb[:, :],
                )

            # ---------------------------------------------------------------
            # filler queue: (emit_fn, mm_cost); popped inside the kt loop
            # ---------------------------------------------------------------
            # (emit_fn, mm_cost, min_slot): min_slot keeps a unit from being
            # emitted before its DMAs plausibly landed -- a premature unit
            # blocks the in-order PE stream and starves the exp pipeline
            fillers = deque()
            for q in range(4):                      # KyT[0] first (kt=8 deadline)
                fillers.append((lambda q=q: emit_qk_quarter(2, 0, q), 4, q // 2))
            for i, sg in enumerate(range(4)):       # V from x
                for q in range(4):
                    fillers.append(
                        (lambda sg=sg, q=q: emit_v_quarter(False, sg, q), 4,
                         3 + 2 * i + q // 2))
            for pi in range(3):                     # QK for t=1
                for q in range(4):
                    fillers.append(
                        (lambda pi=pi, q=q: emit_qk_quarter(pi, 1, q), 4, 0))
            for i, sg in enumerate(range(4)):       # V from y
                for q in range(4):
                    fillers.append(
                        (lambda sg=sg, q=q: emit_v_quarter(True, sg, q), 4,
                         8 + 2 * i + q // 2))
            for ft in (2, 3):                       # QK for t=2,3
                for pi in range(3):
                    for q in range(4):
                        fillers.append(
                            (lambda pi=pi, ft=ft, q=q: emit_qk_quarter(pi, ft, q),
                             4, 0))
            late_fillers = deque()                  # gated: need oT[0..3] half 0
            for m in range(8):
                late_fillers.append(
                    (lambda m=m: emit_op_group(m, 0, scps if m % 2 else pjps),
                     5, 0))

            # ---------------------------------------------------------------
            # attention blocks
            # ---------------------------------------------------------------
            blocks = [(t, qt) for t in range(4) for qt in range(2)]

            # per-block AV state
            pending_av = []        # (block_idx, kt, p2, av_tile, t), oldest first
            av_tiles = {}
            avs_flushed = {}       # block_idx -> count
            av_started = set()     # block_idx whose first AV pair was emitted
            block_recip = {}       # block_idx -> recip tile (emitted with dsum)
            mul_done = set()
            credit = -float(POP_START_SLOT) * FILLER_MM_PER_SLOT
            exp_count = 0
            av_pairs_flushed_total = 0

            def flush_avs(max_pairs):
                # out-of-order within the V-readiness constraint: a blocked
                # V-y tile must not wedge flushable x-side AVs (deadlock via
                # p2-pool exhaustion otherwise).  Oldest blocks first.
                nonlocal av_pairs_flushed_total
                n = 0
                i = 0
                while i < len(pending_av) and n < max_pairs:
                    bi, kt, p2t, avt, t = pending_av[i]
                    if not v_ready[kt]:
                        i += 1
                        continue
                    pending_av.pop(i)
                    first = bi not in av_started
                    av_started.add(bi)
                    last = avs_flushed.get(bi, 0) == 15
                    for hh in range(2):
                        nc.tensor.matmul(
                            avt[hh * 64:(hh + 1) * 64, :],
                            V[kt][:, 2 * t + hh, :],
                            p2t[:, hh * 512:(hh + 1) * 512],
                            start=first,
                            stop=last,
                            skip_group_check=True,
                        )
                    avs_flushed[bi] = avs_flushed.get(bi, 0) + 1
                    av_pairs_flushed_total += 1
                    n += 1
                    if avs_flushed[bi] == 16:
                        maybe_emit_mul(bi)

            def maybe_emit_mul(bi):
                if bi in mul_done:
                    return
                if avs_flushed.get(bi, 0) == 16 and bi in block_recip:
                    t, qt = blocks[bi]
                    nc.vector.tensor_mul(
                        out=oT[t][:, qt * 512:(qt + 1) * 512],
                        in0=av_tiles[bi][:, :],
                        in1=block_recip[bi][:, :],
                    )
                    mul_done.add(bi)

            def emit_dsum(bi, accs):
                dsum = scps.tile([128, 512], FP32, tag="sc", name=f"dsum{bi}")
                for hh in range(2):
                    for j, a in enumerate(accs):
                        nc.tensor.matmul(
                            dsum[hh * 64:(hh + 1) * 64, :],
                            ones64[:, :],
                            a[:, hh * 512:(hh + 1) * 512],
                            start=(j == 0),
                            stop=(j == 1),
                            skip_group_check=True,
                        )
                # fast copy releases the PSUM slot; the ~3.3us iterative
                # reciprocal must not gate the score-tile rotation
                dsb = rpool.tile([128, 512], FP32, tag="dsb", name=f"dsb{bi}")
                nc.vector.tensor_copy(out=dsb[:, :], in_=dsum[:, :])
                rec = rpool.tile([128, 512], FP32, tag="rec", name=f"rec{bi}")
                nc.vector.reciprocal(out=rec[:, :], in_=dsb[:, :])
                block_recip[bi] = rec
                maybe_emit_mul(bi)

            # ~36 dummy matmuls (~3.6us busy) to trip the HAM clock gate to
            # 8/8 while the first DMAs are still streaming
            wps = pjps.tile([128, 1024], FP32, tag="pj", name="warmps")
            for i in range(36):
                nc.tensor.matmul(
                    wps[:, 0:128],
                    warm[:, :],
                    warm[:, :],
                    start=(i == 0),
                    stop=(i == 35),
                )

            # inline: QT[0], KxT[0] — block 0 cannot start without them
            for pi in range(2):
                for q in range(4):
                    emit_qk_quarter(pi, 0, q)

            acc_pair = {}
            prev_acc = None     # (block_idx, acc tiles) awaiting dsum
            for bi, (t, qt) in enumerate(blocks):
                avt = avps.tile([128, 512], FP32, tag="av", name=f"av{bi}")
                av_tiles[bi] = avt
                acc = accpool.tile([128, 1024], BF16, tag="acc", name=f"acc{bi}")
                accg = accpool.tile([128, 1024], BF16, tag="accg", name=f"accg{bi}")
                for kt in range(16):
                    KT = KxT[t] if kt < 8 else KyT[t]
                    ks = (kt % 8) * 128
                    sc = scps.tile([128, 1024], FP32, tag="sc", name="sc")
                    for hh in range(2):
                        nc.tensor.matmul(
                            sc[:, hh * 512:(hh + 1) * 512],
                            KT[hh * 64:(hh + 1) * 64, ks:ks + 128],
                            QT[t][hh * 64:(hh + 1) * 64, qt * 512:(qt + 1) * 512],
                            start=True,
                            stop=True,
                        )
                    p2 = ppool.tile([128, 1024], BF16, tag="p", name="p")
                    nc.scalar.activation(out=p2[:, :], in_=sc[:, :], func=EXP)
                    exp_count += 1
                    eng, at = (nc.vector, acc) if kt % 2 == 0 else (nc.gpsimd, accg)
                    if kt < 2:
                        eng.tensor_copy(out=at[:, :], in_=p2[:, :])
                    else:
                        eng.tensor_add(out=at[:, :], in0=at[:, :], in1=p2[:, :])
                    pending_av.append((bi, kt, p2, avt, t))
                    acc_pair[bi] = (acc, accg)

                    # previous block's denominator, once its acc chain is done
                    if kt == 2 and prev_acc is not None:
                        emit_dsum(*prev_acc)
                        prev_acc = None

                    # paced fillers (out-projection half 0 only once every
                    # qt=0 block has normalized, else its MMs could deadlock
                    # against the yet-unemitted mul on the in-order streams)
                    slot = bi * 16 + kt
                    rate = FILLER_MM_PER_SLOT if slot < 32 else 3.4
                    credit = min(credit + rate, 9.0)

                    def pick_src(slot=slot):
                        if fillers:
                            return fillers if fillers[0][2] <= slot else None
                        if late_fillers and {0, 2, 4, 6} <= mul_done:
                            return late_fillers
                        return None

                    src = pick_src()
                    while src and credit >= src[0][1]:
                        fn, cost, _ms = src.popleft()
                        fn()
                        credit -= cost
                        src = pick_src()

                    # AV flushes: keep p2 pool pressure bounded; free-run when
                    # fillers are done
                    live = exp_count - av_pairs_flushed_total
                    if not fillers:
                        flush_avs(AV_FLUSH_PER_SLOT + 2)
                    elif live > P2_TARGET_LIVE:
                        flush_avs(AV_FLUSH_PER_SLOT)
                prev_acc = (bi, acc_pair[bi])

            # tail: remaining denominator, AVs, muls, out-projection
            emit_dsum(*prev_acc)
            flush_avs(10 ** 9)
            while fillers:
                fn, _ = fillers.popleft()
                fn()
            while late_fillers:
                fn, _ = late_fillers.popleft()
                fn()
            for bi in range(8):
                maybe_emit_mul(bi)
            for m in range(8):
                emit_op_group(m, 1, scps if m % 2 else pjps)

    _spill_excess_waits(nc)
    return nc


_NC = None


def _get_program():
    global _NC
    if _NC is None:
        _NC = _build_program()
    return _NC


# ---------------------------------------------------------------------------
# host wrapper
# ---------------------------------------------------------------------------

def _prep_in_maps(x, y, W_Kx, b_Kx, W_Qx, b_Qx, W_Vx, b_Vx, W_Ky, b_Ky,
                  W_Vy, b_Vy, W_out, b_out):
    f32 = np.float32
    bf16 = ml_dtypes.bfloat16
    in_maps = []
    for c in range(NCORES):
        b = c // 2
        g = c % 2
        gs = slice(FS * g, FS * (g + 1))
        m = {
            "xT": np.ascontiguousarray(np.asarray(x[b], f32).T).astype(bf16),
            "yT": np.ascontiguousarray(np.asarray(y[b], f32).T).astype(bf16),
            "wq": np.ascontiguousarray((np.asarray(W_Qx, f32)[gs, :] / 8.0).T).astype(bf16),
            "wkx": np.ascontiguousarray(np.asarray(W_Kx, f32)[gs, :].T).astype(bf16),
            "wky": np.ascontiguousarray(np.asarray(W_Ky, f32)[gs, :].T).astype(bf16),
            "wvx": np.ascontiguousarray(np.asarray(W_Vx, f32)[gs, :].T).astype(bf16),
            "wvy": np.ascontiguousarray(np.asarray(W_Vy, f32)[gs, :].T).astype(bf16),
            "wo": np.ascontiguousarray(np.asarray(W_out, f32)[:, gs].T).astype(bf16),
            "bq": np.ascontiguousarray(
                (np.asarray(b_Qx, f32)[gs] / 8.0).reshape(4, 128).T),
            "bkx": np.ascontiguousarray(np.asarray(b_Kx, f32)[gs].reshape(4, 128).T),
            "bky": np.ascontiguousarray(np.asarray(b_Ky, f32)[gs].reshape(4, 128).T),
            "bvx_bc": np.ascontiguousarray(np.asarray(b_Vx, f32)[gs].reshape(1, FS)),
            "bvy_bc": np.ascontiguousarray(np.asarray(b_Vy, f32)[gs].reshape(1, FS)),
        }
        in_maps.append(m)
    return in_maps


def _assemble(results, b_out):
    B = 4
    out = np.empty((B, S, DIM), np.float32)
    bo = np.asarray(b_out, np.float32)
    for b in range(B):
        acc = results[2 * b]["outT"] + results[2 * b + 1]["outT"]
        out[b] = acc.T + bo
    return out


def kernel(**inputs):
    nc = _get_program()
    in_maps = _prep_in_maps(**inputs)
    last_err = None
    for _attempt in range(3):
        try:
            res = run_bass_kernel_spmd(nc, in_maps, core_ids=list(range(NCORES)))
            return _assemble(res.results, inputs["b_out"])
        except Exception as e:  # transient NRT_EXEC_UNIT_UNRECOVERABLE after fresh compile
            last_err = e
            import time as _time
            _time.sleep(2.0)
    raise last_err


def kernel_traced(trace_cores=None, **inputs):
    """Same as kernel() but returns (out, BassKernelResults) with NTFF trace."""
    _register_ntff_hook()
    nc = _get_program()
    in_maps = _prep_in_maps(**inputs)
    res = run_bass_kernel_spmd(
        nc, in_maps, core_ids=list(range(NCORES)), trace=True,
        trace_cores=trace_cores or [0],
    )
    return _assemble(res.results, inputs["b_out"]), res


# revision 24
# speedup vs baseline: 1.2699x; 1.2699x over previous
"""MultiHeadCrossAttention kernel for 8 Trainium2 NeuronCores.

Problem (hardcoded): B=4, Sx=Sy=1024, DIM=1024, H=16, Dh=64, fp32.
  Q = x@W_Qx.T+b_Qx ; K = cat(x@W_Kx.T+b_Kx, y@W_Ky.T+b_Ky) per head
  V = cat(x@W_Vx.T+b_Vx, y@W_Vy.T+b_Vy) ; out = softmax(QK^T/8)V @ W_out.T + b_out

Sharding: core c -> (batch b = c//2, head-group g = c%2 of 8 heads).
Each core computes its batch's attention for its 8 heads plus the partial
out-projection over its 512 features; host sums the two partials per batch
and adds b_out (the "all-reduce after to_out", done in the gather).

Schedule (v2): the ScalarE exp stream (128 x [128,1024] activations ~ 148us)
is the critical path.  Attention blocks start as early as possible (~12us);
all projection work (V, QK for t>=1, out-projection half 0) is emitted as
paced "filler" units inside the attention kt loop so it hides under the
exp stream.  AV matmuls are col-packed two heads per PSUM bank
(tile_position (0,0)/(0,64), M=64 each); the softmax denominator is
accumulated on VectorE (fp32) and turned into a 64-row broadcast via a
ones[128,64] matmul pair, so no ones-column rides the AV matmul.
reciprocal_approx_fast replaces the slow iterative divide.
"""

import os
import sys

os.environ.setdefault("MYCRO_LOCAL_CACHE", "1")
if "/opt/trn_rl_repo" not in sys.path:
    sys.path.insert(0, "/opt/trn_rl_repo")

from collections import deque

import ml_dtypes
import numpy as np

import concourse.bass as bass
import concourse.mybir as mybir
import concourse.tile as tile
from concourse import bass_utils
from concourse.bass_utils import run_bass_kernel_spmd

FP32 = mybir.dt.float32
FP32R = mybir.dt.float32r
BF16 = mybir.dt.bfloat16

DIM = 1024
H = 16          # total heads
HG = 8          # heads per core (head-group)
DH = 64
S = 1024        # Sx = Sy
FS = 512        # feature slice per core (HG * DH)
NCORES = 8

# ---------------------------------------------------------------------------
# harness patches (this snapshot's Tile emits >1 wait per instruction in a
# few places; HW instructions hold one wait)
# ---------------------------------------------------------------------------

def _patched_drain_and_barrier(self, tick_clock, wait_clock):
    from bass_rust import ScopedClock

    nc = self.nc
    drain_inst = nc.sync.drain()
    wait_clock.add_sem_waits(
        drain_inst.ins, ScopedClock({None: tick_clock.global_clock})
    )
    si = drain_inst.ins.sync_info
    waits = list(si.on_wait)
    if len(waits) > 1:
        del si.on_wait[1:]
        for w in waits[1:]:
            nop = nc.sync.nop(nofuse=True, hint="drain_wait_spill")
            if nop.ins.sync_info is None:
                nop.ins.sync_info = mybir.SyncInfo(on_wait=[], on_update=[])
            nop.ins.sync_info.on_wait.append(w)

    nc.all_engine_barrier()
    assert self.sems is not None
    popped = nc._tile_sem_poison_stack.pop()
    assert popped is self._sem_poison
    nc.clear_and_free_semaphores(list(self.sems.allocated().values()))
    nc.all_engine_barrier()


def _spill_excess_waits(nc):
    n = 0
    for fn in nc.m.functions:
        for bb in fn.blocks:
            new_insts = []
            for inst in bb.instructions:
                si = getattr(inst, "sync_info", None)
                cap = 2 if isinstance(inst, mybir.InstEventSemaphore) else 1
                if si is not None and si.on_wait and len(si.on_wait) > cap:
                    extras = list(si.on_wait[cap:])
                    del si.on_wait[cap:]
                    for w in extras:
                        new_insts.append(
                            mybir.InstNoOp(
                                name=f"wspill-{nc.next_id()}",
                                engine=inst.engine,
                                ins=[],
                                outs=[],
                                sync_info=mybir.SyncInfo(on_wait=[w], on_update=[]),
                            )
                        )
                        n += 1
                new_insts.append(inst)
            bb.instructions[:] = new_insts
    return n


tile.TileContext._drain_and_barrier = _patched_drain_and_barrier

if os.environ.get("ENABLE_LDW_OPT") == "1":
    _orig_run_command = bass_utils.run_command

    def _run_command_ldw(argv, **kwargs):
        if isinstance(argv, list):
            argv = ["--enable-ldw-opt=true" if a == "--enable-ldw-opt=false" else a
                    for a in argv]
        return _orig_run_command(argv, **kwargs)

    bass_utils.run_command = _run_command_ldw
bass_utils.upload_artifacts = lambda tmpdir: tmpdir  # no S3 in container


def _register_ntff_hook():
    """Best-effort: enables trace=True runs (used by test harness only)."""
    try:
        import types

        try:
            from antenv.axon_hooks import set_axon_ntff_profile_hook
        except ImportError:
            # this snapshot's antenv lacks axon_hooks; synthesize the
            # two-function hook registry bass_utils expects
            import antenv

            mod = types.ModuleType("antenv.axon_hooks")
            mod._HOOK = None

            def set_axon_ntff_profile_hook(h, _mod=mod):
                _mod._HOOK = h

            def get_axon_ntff_profile_hook(_mod=mod):
                return _mod._HOOK

            mod.set_axon_ntff_profile_hook = set_axon_ntff_profile_hook
            mod.get_axon_ntff_profile_hook = get_axon_ntff_profile_hook
            sys.modules["antenv.axon_hooks"] = mod
            antenv.axon_hooks = mod
        sys.path.insert(0, "/root/.axon_site")
        from trn_agent_boot.trn_boot import _ntff_profile_via_ctypes

        set_axon_ntff_profile_hook(
            _ntff_profile_via_ctypes("/opt/axon/libaxon_pjrt.so")
        )
    except Exception:
        pass


# ---------------------------------------------------------------------------
# device program (identical on all 8 cores; per-core data differs)
# ---------------------------------------------------------------------------

FILLER_MM_PER_SLOT = 4.5   # paced filler emission budget (MMs per exp slot)
POP_START_SLOT = 4         # no filler pops before this global exp slot
P2_BUFS = 22               # softmax-weight tiles buffered in SBUF
P2_TARGET_LIVE = 14        # defer AV flush until this many p2 tiles are live
AV_FLUSH_PER_SLOT = 2      # max AV pairs flushed per slot when over target


def _build_program():
    nc = bass.Bass()

    xT = nc.declare_dram_parameter("xT", [DIM, S], BF16, isOutput=False)
    yT = nc.declare_dram_parameter("yT", [DIM, S], BF16, isOutput=False)
    wq = nc.declare_dram_parameter("wq", [DIM, FS], BF16, isOutput=False)
    wkx = nc.declare_dram_parameter("wkx", [DIM, FS], BF16, isOutput=False)
    wky = nc.declare_dram_parameter("wky", [DIM, FS], BF16, isOutput=False)
    wvx = nc.declare_dram_parameter("wvx", [DIM, FS], BF16, isOutput=False)
    wvy = nc.declare_dram_parameter("wvy", [DIM, FS], BF16, isOutput=False)
    wo = nc.declare_dram_parameter("wo", [FS, DIM], BF16, isOutput=False)
    bq = nc.declare_dram_parameter("bq", [128, 4], FP32, isOutput=False)
    bkx = nc.declare_dram_parameter("bkx", [128, 4], FP32, isOutput=False)
    bky = nc.declare_dram_parameter("bky", [128, 4], FP32, isOutput=False)
    bvx_bc = nc.declare_dram_parameter("bvx_bc", [1, FS], FP32, isOutput=False)
    bvy_bc = nc.declare_dram_parameter("bvy_bc", [1, FS], FP32, isOutput=False)
    outT = nc.declare_dram_parameter("outT", [DIM, S], FP32, isOutput=True)

    EXP = mybir.ActivationFunctionType.Exp

    with tile.TileContext(nc) as tc:
        import contextlib

        with contextlib.ExitStack() as ctx:
            # SBUF pools
            apool = ctx.enter_context(tc.tile_pool(name="apool", bufs=16))
            wpool = ctx.enter_context(tc.tile_pool(name="wpool", bufs=40))
            wopool = ctx.enter_context(tc.tile_pool(name="wopool", bufs=4))
            qkpool = ctx.enter_context(tc.tile_pool(name="qkpool", bufs=12))
            vpool = ctx.enter_context(tc.tile_pool(name="vpool", bufs=16))
            ppool = ctx.enter_context(tc.tile_pool(name="ppool", bufs=P2_BUFS))
            accpool = ctx.enter_context(tc.tile_pool(name="accpool", bufs=2))
            rpool = ctx.enter_context(tc.tile_pool(name="rpool", bufs=3))
            otpool = ctx.enter_context(tc.tile_pool(name="otpool", bufs=4))
            osbpool = ctx.enter_context(tc.tile_pool(name="osbpool", bufs=2))
            cpool = ctx.enter_context(tc.tile_pool(name="cpool", bufs=1))
            # PSUM pools: 2*2 + 2*1 + 1*2 = 8 banks
            scps = ctx.enter_context(tc.tile_pool(name="scps", bufs=2, space="PSUM"))
            avps = ctx.enter_context(tc.tile_pool(name="avps", bufs=2, space="PSUM"))
            pjps = ctx.enter_context(tc.tile_pool(name="pjps", bufs=1, space="PSUM"))

            # ---- constants + ACT table warm-up ----
            ones64 = cpool.tile([128, 64], BF16, tag="ones64")
            nc.vector.memset(ones64[:, :], 1.0)
            dwi = cpool.tile([128, 8], FP32, tag="dwi")
            dwo = cpool.tile([128, 8], BF16, tag="dwo")
            nc.vector.memset(dwi[:, :], 0.0)
            # loads the exp table set (~2.7us) while DMAs stream in
            nc.scalar.activation(out=dwo[:, :], in_=dwi[:, :], func=EXP)
            warm = cpool.tile([128, 128], BF16, tag="warm")
            nc.vector.memset(warm[:, :], 0.0)

            bq_sb = cpool.tile([128, 4], FP32, tag="bq")
            bkx_sb = cpool.tile([128, 4], FP32, tag="bkx")
            bky_sb = cpool.tile([128, 4], FP32, tag="bky")
            bvx_sb = cpool.tile([128, FS], FP32, tag="bvx")
            bvy_sb = cpool.tile([128, FS], FP32, tag="bvy")

            def _bcast_ap(h):
                return bass.AP(
                    tensor=h[:, :].tensor, offset=h[:, :].offset,
                    ap=[[0, 128]] + [list(a) for a in h[:, :].ap[1:]],
                )

            nc.sync.dma_start(out=bq_sb, in_=bq[:, :])
            nc.sync.dma_start(out=bkx_sb, in_=bkx[:, :])
            nc.sync.dma_start(out=bky_sb, in_=bky[:, :])
            nc.gpsimd.dma_start(out=bvx_sb, in_=_bcast_ap(bvx_bc))
            nc.gpsimd.dma_start(out=bvy_sb, in_=_bcast_ap(bvy_bc))

            # ---- big DMAs, in consumption-priority order ----
            def load_rows(dst_pool, dram, n, width, nm, tag, eng=None):
                eng = eng or nc.sync
                out = []
                for i in range(n):
                    t = dst_pool.tile([128, width], BF16, tag=tag, name=f"{nm}{i}")
                    eng.dma_start(out=t, in_=dram[i * 128:(i + 1) * 128, :])
                    out.append(t)
                return out

            # two DMA queues in parallel: weights on sync, activations on
            # gpsimd -- first score needs wq+wkx+xT, which lands ~2x sooner
            xt = load_rows(apool, xT, 8, S, "xt", "act", nc.gpsimd)
            yt = load_rows(apool, yT, 8, S, "yt", "act", nc.gpsimd)
            wq_sb = load_rows(wpool, wq, 8, FS, "wq", "w")
            wkx_sb = load_rows(wpool, wkx, 8, FS, "wkx", "w")
            wky_sb = load_rows(wpool, wky, 8, FS, "wky", "w")
            wvx_sb = load_rows(wpool, wvx, 8, FS, "wvx", "w")
            wvy_sb = load_rows(wpool, wvy, 8, FS, "wvy", "w")
            wo_sb = load_rows(wopool, wo, 4, S, "wo", "wo")

            # ---- persistent result tiles ----
            QT = [qkpool.tile([128, S], BF16, tag="qk", name=f"QT{i}") for i in range(4)]
            KxT = [qkpool.tile([128, S], BF16, tag="qk", name=f"KxT{i}") for i in range(4)]
            KyT = [qkpool.tile([128, S], BF16, tag="qk", name=f"KyT{i}") for i in range(4)]
            V = [vpool.tile([128, HG, DH], BF16, tag="v", name=f"V{i}") for i in range(16)]
            oT = [otpool.tile([128, S], BF16, tag="ot", name=f"oT{i}") for i in range(4)]

            qk_w = [wq_sb, wkx_sb, wky_sb]
            qk_act = [xt, xt, yt]
            qk_bias = [bq_sb, bkx_sb, bky_sb]
            qk_dst = [QT, KxT, KyT]

            # ---------------------------------------------------------------
            # group emitters (each emits a quarter = 4 MMs; q==3 evacuates)
            # ---------------------------------------------------------------
            qk_ps = {}

            def emit_qk_quarter(pi, ft, q):
                key = (pi, ft)
                if key not in qk_ps:
                    qk_ps[key] = pjps.tile(
                        [128, 1024], FP32, tag="pj", name=f"qkps{pi}_{ft}"
                    )
                ps = qk_ps[key]
                for ct in (2 * q, 2 * q + 1):
                    for h2 in range(2):
                        nc.tensor.matmul(
                            ps[:, h2 * 512:(h2 + 1) * 512],
                            qk_w[pi][ct][:, ft * 128:(ft + 1) * 128],
                            qk_act[pi][ct][:, h2 * 512:(h2 + 1) * 512],
                            start=(ct == 0),
                            stop=(ct == 7),
                        )
                if q == 3:
                    nc.vector.tensor_scalar_add(
                        out=qk_dst[pi][ft][:, :],
                        in0=ps[:, :],
                        scalar1=qk_bias[pi][:, ft:ft + 1],
                    )
                    del qk_ps[key]

            v_ps = {}
            v_ready = [False] * 16

            def emit_v_quarter(src_is_y, sg, q):
                key = (src_is_y, sg)
                if key not in v_ps:
                    v_ps[key] = pjps.tile(
                        [128, 1024], FP32, tag="pj", name=f"vps{int(src_is_y)}_{sg}"
                    )
                ps = v_ps[key]
                act = yt if src_is_y else xt
                w_sb = wvy_sb if src_is_y else wvx_sb
                bias_sb = bvy_sb if src_is_y else bvx_sb
                base = 8 if src_is_y else 0
                for ct in (2 * q, 2 * q + 1):
                    for half in range(2):
                        st = 2 * sg + half
                        nc.tensor.matmul(
                            ps[:, half * 512:(half + 1) * 512],
                            act[ct][:, st * 128:(st + 1) * 128],
                            w_sb[ct][:, :],
                            start=(ct == 0),
                            stop=(ct == 7),
                        )
                if q == 3:
                    for half in range(2):
                        st = 2 * sg + half
                        vt = V[base + st]
                        nc.vector.tensor_add(
                            out=vt[:, :, :],
                            in0=ps[:, half * 512:(half + 1) * 512].rearrange(
                                "p (h d) -> p h d", h=HG),
                            in1=bias_sb[:, :].rearrange("p (h d) -> p h d", h=HG),
                        )
                        v_ready[base + st] = True
                    del v_ps[key]

            def emit_op_group(m, half, pool):
                ps = pool.tile([128, 512], FP32,
                               tag="pj" if pool is pjps else "sc",
                               name=f"op{m}_{half}")
                for ft in range(4):
                    nc.tensor.matmul(
                        ps[:, :],
                        wo_sb[ft][:, m * 128:(m + 1) * 128],
                        oT[ft][:, half * 512:(half + 1) * 512],
                        start=(ft == 0),
                        stop=(ft == 3),
                    )
                osb = osbpool.tile([128, 512], FP32, tag="osb", name="osb")
                nc.vector.tensor_copy(out=osb[:, :], in_=ps[:, :])
                nc.sync.dma_start(
                    out=outT[m * 128:(m + 1) * 128, half * 512:(half + 1) * 512],
                    in_=osb[:, :],
                )

            # ---------------------------------------------------------------
            # filler queue: (emit_fn, mm_cost); popped inside the kt loop
            # ---------------------------------------------------------------
            # (emit_fn, mm_cost, min_slot): min_slot keeps a unit from being
            # emitted before its DMAs plausibly landed -- a premature unit
            # blocks the in-order PE stream and starves the exp pipeline
            fillers = deque()
            for q in range(4):                      # KyT[0] first (kt=8 deadline)
                fillers.append((lambda q=q: emit_qk_quarter(2, 0, q), 4, q // 2))
            for i, sg in enumerate(range(4)):       # V from x
                for q in range(4):
                    fillers.append(
                        (lambda sg=sg, q=q: emit_v_quarter(False, sg, q), 4,
                         3 + 2 * i + q // 2))
            for pi in range(3):                     # QK for t=1
                for q in range(4):
                    fillers.append(
                        (lambda pi=pi, q=q: emit_qk_quarter(pi, 1, q), 4, 0))
            for i, sg in enumerate(range(4)):       # V from y
                for q in range(4):
                    fillers.append(
                        (lambda sg=sg, q=q: emit_v_quarter(True, sg, q), 4,
                         8 + 2 * i + q // 2))
            for ft in (2, 3):                       # QK for t=2,3
                for pi in range(3):
                    for q in range(4):
                        fillers.append(
                            (lambda pi=pi, ft=ft, q=q: emit_qk_quarter(pi, ft, q),
                             4, 0))
            late_fillers = deque()                  # gated: need oT[0..3] half 0
            for m in range(8):
                late_fillers.append(
                    (lambda m=m: emit_op_group(m, 0, scps if m % 2 else pjps),
                     5, 0))

            # ---------------------------------------------------------------
            # attention blocks
            # ---------------------------------------------------------------
            blocks = [(t, qt) for t in range(4) for qt in range(2)]

            # per-block AV state
            pending_av = []        # (block_idx, kt, p2, av_tile, t), oldest first
            av_tiles = {}
            avs_flushed = {}       # block_idx -> count
            av_started = set()     # block_idx whose first AV pair was emitted
            block_recip = {}       # block_idx -> recip tile (emitted with dsum)
            mul_done = set()
            credit = -float(POP_START_SLOT) * FILLER_MM_PER_SLOT
            exp_count = 0
            av_pairs_flushed_total = 0

            def flush_avs(max_pairs):
                # out-of-order within the V-readiness constraint: a blocked
                # V-y tile must not wedge flushable x-side AVs (deadlock via
                # p2-pool exhaustion otherwise).  Oldest blocks first.
                nonlocal av_pairs_flushed_total
                n = 0
                i = 0
                while i < len(pending_av) and n < max_pairs:
                    bi, kt, p2t, avt, t = pending_av[i]
                    if not v_ready[kt]:
                        i += 1
                        continue
                    pending_av.pop(i)
                    first = bi not in av_started
                    av_started.add(bi)
                    last = avs_flushed.get(bi, 0) == 15
                    for hh in range(2):
                        nc.tensor.matmul(
                            avt[hh * 64:(hh + 1) * 64, :],
                            V[kt][:, 2 * t + hh, :],
                            p2t[:, hh * 512:(hh + 1) * 512],
                            start=first,
                            stop=last,
                            skip_group_check=True,
                        )
                    avs_flushed[bi] = avs_flushed.get(bi, 0) + 1
                    av_pairs_flushed_total += 1
                    n += 1
                    if avs_flushed[bi] == 16:
                        maybe_emit_mul(bi)

            def maybe_emit_mul(bi):
                if bi in mul_done:
                    return
                if avs_flushed.get(bi, 0) == 16 and bi in block_recip:
                    t, qt = blocks[bi]
                    nc.vector.tensor_mul(
                        out=oT[t][:, qt * 512:(qt + 1) * 512],
                        in0=av_tiles[bi][:, :],
                        in1=block_recip[bi][:, :],
                    )
                    mul_done.add(bi)

            def emit_dsum(bi, acc):
                dsum = scps.tile([128, 512], FP32, tag="sc", name=f"dsum{bi}")
                for hh in range(2):
                    nc.tensor.matmul(
                        dsum[hh * 64:(hh + 1) * 64, :],
                        ones64[:, :],
                        acc[:, hh * 512:(hh + 1) * 512],
                        start=True,
                        stop=True,
                        skip_group_check=True,
                    )
                # fast copy releases the PSUM slot; the ~3.3us iterative
                # reciprocal must not gate the score-tile rotation
                dsb = rpool.tile([128, 512], FP32, tag="dsb", name=f"dsb{bi}")
                nc.vector.tensor_copy(out=dsb[:, :], in_=dsum[:, :])
                rec = rpool.tile([128, 512], FP32, tag="rec", name=f"rec{bi}")
                nc.vector.reciprocal(out=rec[:, :], in_=dsb[:, :])
                block_recip[bi] = rec
                maybe_emit_mul(bi)

            # ~36 dummy matmuls (~3.6us busy) to trip the HAM clock gate to
            # 8/8 while the first DMAs are still streaming
            wps = pjps.tile([128, 1024], FP32, tag="pj", name="warmps")
            for i in range(36):
                nc.tensor.matmul(
                    wps[:, 0:128],
                    warm[:, :],
                    warm[:, :],
                    start=(i == 0),
                    stop=(i == 35),
                )

            # inline: QT[0], KxT[0] — block 0 cannot start without them
            for pi in range(2):
                for q in range(4):
                    emit_qk_quarter(pi, 0, q)

            prev_acc = None     # (block_idx, acc tile) awaiting dsum
            for bi, (t, qt) in enumerate(blocks):
                avt = avps.tile([128, 512], FP32, tag="av", name=f"av{bi}")
                av_tiles[bi] = avt
                acc = accpool.tile([128, 1024], BF16, tag="acc", name=f"acc{bi}")
                for kt in range(16):
                    KT = KxT[t] if kt < 8 else KyT[t]
                    ks = (kt % 8) * 128
                    sc = scps.tile([128, 1024], FP32, tag="sc", name="sc")
                    for hh in range(2):
                        nc.tensor.matmul(
                            sc[:, hh * 512:(hh + 1) * 512],
                            KT[hh * 64:(hh + 1) * 64, ks:ks + 128],
                            QT[t][hh * 64:(hh + 1) * 64, qt * 512:(qt + 1) * 512],
                            start=True,
                            stop=True,
                        )
                    p2 = ppool.tile([128, 1024], BF16, tag="p", name="p")
                    nc.scalar.activation(out=p2[:, :], in_=sc[:, :], func=EXP)
                    exp_count += 1
                    if kt == 0:
                        nc.vector.tensor_copy(out=acc[:, :], in_=p2[:, :])
                    else:
                        nc.vector.tensor_add(out=acc[:, :], in0=acc[:, :], in1=p2[:, :])
                    pending_av.append((bi, kt, p2, avt, t))

                    # previous block's denominator, once its acc chain is done
                    if kt == 2 and prev_acc is not None:
                        emit_dsum(*prev_acc)
                        prev_acc = None

                    # paced fillers (out-projection half 0 only once every
                    # qt=0 block has normalized, else its MMs could deadlock
                    # against the yet-unemitted mul on the in-order streams)
                    slot = bi * 16 + kt
                    rate = FILLER_MM_PER_SLOT if slot < 32 else 3.4
                    credit = min(credit + rate, 9.0)

                    def pick_src(slot=slot):
                        if fillers:
                            return fillers if fillers[0][2] <= slot else None
                        if late_fillers and {0, 2, 4, 6} <= mul_done:
                            return late_fillers
                        return None

                    src = pick_src()
                    while src and credit >= src[0][1]:
                        fn, cost, _ms = src.popleft()
                        fn()
                        credit -= cost
                        src = pick_src()

                    # AV flushes: keep p2 pool pressure bounded; free-run when
                    # fillers are done
                    live = exp_count - av_pairs_flushed_total
                    if not fillers:
                        flush_avs(AV_FLUSH_PER_SLOT + 2)
                    elif live > P2_TARGET_LIVE:
                        flush_avs(AV_FLUSH_PER_SLOT)
                prev_acc = (bi, acc)

            # tail: remaining denominator, AVs, muls, out-projection
            emit_dsum(*prev_acc)
            flush_avs(10 ** 9)
            while fillers:
                fn, _ = fillers.popleft()
                fn()
            while late_fillers:
                fn, _ = late_fillers.popleft()
                fn()
            for bi in range(8):
                maybe_emit_mul(bi)
            for m in range(8):
                emit_op_group(m, 1, scps if m % 2 else pjps)

    _spill_excess_waits(nc)
    return nc


_NC = None


def _get_program():
    global _NC
    if _NC is None:
        _NC = _build_program()
    return _NC


# ---------------------------------------------------------------------------
# host wrapper
# ---------------------------------------------------------------------------

def _prep_in_maps(x, y, W_Kx, b_Kx, W_Qx, b_Qx, W_Vx, b_Vx, W_Ky, b_Ky,
                  W_Vy, b_Vy, W_out, b_out):
    f32 = np.float32
    bf16 = ml_dtypes.bfloat16
    in_maps = []
    for c in range(NCORES):
        b = c // 2
        g = c % 2
        gs = slice(FS * g, FS * (g + 1))
        m = {
            "xT": np.ascontiguousarray(np.asarray(x[b], f32).T).astype(bf16),
            "yT": np.ascontiguousarray(np.asarray(y[b], f32).T).astype(bf16),
            "wq": np.ascontiguousarray((np.asarray(W_Qx, f32)[gs, :] / 8.0).T).astype(bf16),
            "wkx": np.ascontiguousarray(np.asarray(W_Kx, f32)[gs, :].T).astype(bf16),
            "wky": np.ascontiguousarray(np.asarray(W_Ky, f32)[gs, :].T).astype(bf16),
            "wvx": np.ascontiguousarray(np.asarray(W_Vx, f32)[gs, :].T).astype(bf16),
            "wvy": np.ascontiguousarray(np.asarray(W_Vy, f32)[gs, :].T).astype(bf16),
            "wo": np.ascontiguousarray(np.asarray(W_out, f32)[:, gs].T).astype(bf16),
            "bq": np.ascontiguousarray(
                (np.asarray(b_Qx, f32)[gs] / 8.0).reshape(4, 128).T),
            "bkx": np.ascontiguousarray(np.asarray(b_Kx, f32)[gs].reshape(4, 128).T),
            "bky": np.ascontiguousarray(np.asarray(b_Ky, f32)[gs].reshape(4, 128).T),
            "bvx_bc": np.ascontiguousarray(np.asarray(b_Vx, f32)[gs].reshape(1, FS)),
            "bvy_bc": np.ascontiguousarray(np.asarray(b_Vy, f32)[gs].reshape(1, FS)),
        }
        in_maps.append(m)
    return in_maps


def _assemble(results, b_out):
    B = 4
    out = np.empty((B, S, DIM), np.float32)
    bo = np.asarray(b_out, np.float32)
    for b in range(B):
        acc = results[2 * b]["outT"] + results[2 * b + 1]["outT"]
        out[b] = acc.T + bo
    return out


def kernel(**inputs):
    nc = _get_program()
    in_maps = _prep_in_maps(**inputs)
    last_err = None
    for _attempt in range(3):
        try:
            res = run_bass_kernel_spmd(nc, in_maps, core_ids=list(range(NCORES)))
            return _assemble(res.results, inputs["b_out"])
        except Exception as e:  # transient NRT_EXEC_UNIT_UNRECOVERABLE after fresh compile
            last_err = e
            import time as _time
            _time.sleep(2.0)
    raise last_err


def kernel_traced(trace_cores=None, **inputs):
    """Same as kernel() but returns (out, BassKernelResults) with NTFF trace."""
    _register_ntff_hook()
    nc = _get_program()
    in_maps = _prep_in_maps(**inputs)
    res = run_bass_kernel_spmd(
        nc, in_maps, core_ids=list(range(NCORES)), trace=True,
        trace_cores=trace_cores or [0],
    )
    return _assemble(res.results, inputs["b_out"]), res
